# revision 40
# baseline (speedup 1.0000x reference)
"""Trainium2 Bass kernel for GQA attention (B=4, T=2048, D=2048, 16 heads / 4 kv groups, RoPE).

Sharding: 8 cores = 4 batches x 2 head-halves. Core c handles batch c//2 and
heads (c%2)*8..+8 with kv groups (c%2)*2..+2.  Per core:
  qkvT projection (channel-major) with RoPE fused into the PSUM eviction
  (chunk-1 q-heads deferred into attention window 0), then a single
  software-pipelined attention loop over (q-chunk, head, score-tile) units:
  S^T tiles via swapped-operand matmul, exp on the scalar engine over mixed
  1024/512-col psum tiles (3 banks), PV in natural [q,d] orientation with a
  ones-column accumulating the softmax denominator (4 banks).  PE transposes
  and row-parallel o_proj matmuls of the previous q-chunk are spliced
  between PV groups (1 bank) so the tensor engine fills the exp-latency
  bubbles; the host sums the two half-core partial [T, D] outputs.
All matmuls in bf16 with fp32 PSUM accumulation.
"""

import numpy as np
import ml_dtypes

BF16 = ml_dtypes.bfloat16

D_MODEL = 2048
NUM_HEADS = 16
QUERY_GROUPS = 4
HEAD_DIM = 128
B = 4
T = 2048
THETA = 10000.0
SCALE = 0.08838834764831845
N_CORES = 8

P = 128
NH = NUM_HEADS // 2          # 8 q heads per core
NG = QUERY_GROUPS // 2       # 2 kv groups per core
QDIM = NH * HEAD_DIM         # 1024
GDIM = NG * HEAD_DIM         # 256
NKT = D_MODEL // P           # 16 contraction tiles over d_model
NTT = T // P                 # 16 tiles over sequence
NCH = T // 512               # 4 chunks of 512 over sequence
NDT = QDIM // P              # 8 head/dim tiles per core

# score-tile schedule per (qc, head): alternating 512-col (B) and 1024-col
# (A) exp tiles; strict alternation keeps the 3-bank mm rotation stall-free.
TILE_SCHED = [("B", [0]), ("A", [1, 2]), ("B", [3]), ("A", [4, 5]),
              ("B", [6]), ("A", [7, 8]), ("B", [9]), ("A", [10, 11]),
              ("B", [12]), ("A", [13, 14]), ("B", [15])]
DEFER_CHUNK = 1              # q-proj chunk spliced into attention window 0


def build_nc(masked: bool):
    import concourse.bacc as bacc
    import concourse.tile as tile
    import concourse.mybir as mybir
    from contextlib import ExitStack

    dt = mybir.dt
    f32 = dt.float32
    bf16 = dt.bfloat16
    AF = mybir.ActivationFunctionType

    nc = bacc.Bacc("TRN2", target_bir_lowering=False, debug=False, num_devices=N_CORES)

    xt = nc.dram_tensor("xt", [D_MODEL, T], bf16, kind="ExternalInput")
    wqk = nc.dram_tensor("wqk", [D_MODEL, QDIM + GDIM], bf16, kind="ExternalInput")
    # chunk-1 q-head weights repacked kt-major per head for the deferred
    # projection: wqk2[p, m*2048 + kt*128 + c] = wqk[kt*128+p, m*128+c]
    wqk2 = nc.dram_tensor("wqk2", [P, NH * D_MODEL], bf16, kind="ExternalInput")
    wv = nc.dram_tensor("wv", [D_MODEL, GDIM], bf16, kind="ExternalInput")
    wo = nc.dram_tensor("wo", [QDIM, D_MODEL], bf16, kind="ExternalInput")
    cosq = nc.dram_tensor("cosq", [P, T], bf16, kind="ExternalInput")
    sinq = nc.dram_tensor("sinq", [P, T], bf16, kind="ExternalInput")
    cosk = nc.dram_tensor("cosk", [P, T], bf16, kind="ExternalInput")
    sink = nc.dram_tensor("sink", [P, T], bf16, kind="ExternalInput")
    if masked:
        maskcol = nc.dram_tensor("maskcol", [P, NTT], f32, kind="ExternalInput")
    out = nc.dram_tensor("out", [T, D_MODEL], bf16, kind="ExternalOutput")

    with tile.TileContext(nc) as tc:
        with ExitStack() as ctx:
            constp = ctx.enter_context(tc.tile_pool(name="const", bufs=1))
            qkT_pool = ctx.enter_context(tc.tile_pool(name="qkT", bufs=NH + NG))
            vnat_pool = ctx.enter_context(tc.tile_pool(name="vnat", bufs=NTT))

            # DVE-memset warmup operand for the PE clock-ramp matmuls
            warmsrc = constp.tile([P, P], bf16, tag="warmsrc", name="warmsrc")
            nc.vector.memset(warmsrc[:], 0.5)
            if masked:
                maskcol_t = constp.tile([P, NTT], f32, tag="maskcol")
                nc.sync.dma_start(out=maskcol_t[:], in_=maskcol[:, :])
            # rope tables for the deferred chunk, kept tiny so the full
            # phase-1 table pool can be released
            tabq2 = {
                "cos": constp.tile([P, 512], bf16, tag="tabq2c", name="tabq2c"),
                "sin": constp.tile([P, 512], bf16, tag="tabq2s", name="tabq2s"),
            }

            # persistent bf16 tensors
            qkT = [qkT_pool.tile([P, T], bf16, tag="qkT", name=f"qkT{i}") for i in range(NH + NG)]
            # v_aug layout per t-tile: [v_g0 | ones | v_g1 | ones] so that the
            # 129-wide slice for group g is contiguous; the ones column makes
            # the PV matmul accumulate the softmax denominator in psum col 128.
            v_aug = [vnat_pool.tile([P, NG * (P + 1)], bf16, tag="vnat", name=f"vaug{i}")
                     for i in range(NTT)]
            for i in range(NTT):
                for g in range(NG):
                    nc.vector.memset(v_aug[i][:, g * (P + 1) + P:g * (P + 1) + P + 1], 1.0)

            # x chunk-1 tiles live here so the deferred q-proj reuses them
            xq_pool = ctx.enter_context(tc.tile_pool(name="xq", bufs=NKT))
            xq_c1 = []

            # ---------------- phase 1: qkv projection + rope -------------
            with ExitStack() as ph1:
                tabp = ph1.enter_context(tc.tile_pool(name="tab", bufs=1))
                wqk_pool = ph1.enter_context(tc.tile_pool(name="wqk", bufs=NKT))
                wv_pool = ph1.enter_context(tc.tile_pool(name="wv", bufs=NKT))
                xc_pool = ph1.enter_context(tc.tile_pool(name="xc", bufs=2 * NKT))
                tmp_pool = ph1.enter_context(tc.tile_pool(name="rtmp", bufs=3))
                pj_pool = ph1.enter_context(
                    tc.tile_pool(name="pj", bufs=4, space="PSUM"))

                # DMA issue plan: sync interleaves xc chunk 0 with wv so the
                # v projection starts ASAP; the scalar engine's DGE issues
                # wqk split into column halves (m 0-4 land before m 5-9 are
                # needed) and then the rope tables.
                wv_t = []
                xc_next = []
                for kt in range(NKT):
                    xtl = xc_pool.tile([P, 512], bf16, tag="xc", name=f"xc{kt}")
                    nc.sync.dma_start(out=xtl[:], in_=xt[kt * P:(kt + 1) * P, 0:512])
                    xc_next.append(xtl)
                    wtl = wv_pool.tile([P, GDIM], bf16, tag="wv", name=f"wvt{kt}")
                    nc.sync.dma_start(out=wtl[:], in_=wv[kt * P:(kt + 1) * P, :])
                    wv_t.append(wtl)
                half_w = (QDIM + GDIM) // 2
                wqk_t = []
                for kt in range(NKT):
                    tl = wqk_pool.tile([P, QDIM + GDIM], bf16, tag="wqk", name=f"wqkt{kt}")
                    nc.scalar.dma_start(out=tl[:, 0:half_w],
                                        in_=wqk[kt * P:(kt + 1) * P, 0:half_w])
                    wqk_t.append(tl)
                tabs = {}
                for nm, tsrc in (("cosq", cosq), ("sinq", sinq),
                                 ("cosk", cosk), ("sink", sink)):
                    tl = tabp.tile([P, T], bf16, tag=nm, name=nm + "_t")
                    nc.scalar.dma_start(out=tl[:], in_=tsrc[:, :])
                    tabs[nm] = tl

                def load_xc(nch):
                    # chunk DEFER_CHUNK lands in the persistent xq pool so
                    # the deferred q-projection can reuse it in phase 2
                    c0 = nch * 512
                    pool = xq_pool if nch == DEFER_CHUNK else xc_pool
                    xc = []
                    for kt in range(NKT):
                        tl = pool.tile([P, 512], bf16, tag=pool.name, name=f"xc{kt}")
                        nc.sync.dma_start(
                            out=tl[:], in_=xt[kt * P:(kt + 1) * P, c0:c0 + 512])
                        xc.append(tl)
                    return xc

                warm = pj_pool.tile([P, P], f32, tag="pj")
                for i in range(40):
                    nc.tensor.matmul(warm[:], lhsT=warmsrc[:], rhs=warmsrc[:],
                                     start=(i == 0), stop=(i == 39))
                wsink = tmp_pool.tile([P, 16], f32, tag="t1")
                nc.vector.tensor_copy(wsink[:], warm[:, 0:16])

                def rope_evict(ps, m, c0, cs, dst):
                    ct = (tabs["cosq"] if m < NH else tabs["cosk"])
                    st = (tabs["sinq"] if m < NH else tabs["sink"])
                    t1 = tmp_pool.tile([P, 512], f32, tag="t1")
                    t2 = tmp_pool.tile([P, 512], f32, tag="t2")
                    hp = P // 2
                    nc.vector.tensor_mul(t1[:], ps[:], ct[:, c0:c0 + cs])
                    nc.vector.tensor_mul(t2[0:hp, :], ps[hp:P, :], st[0:hp, c0:c0 + cs])
                    nc.vector.tensor_mul(t2[hp:P, :], ps[0:hp, :], st[hp:P, c0:c0 + cs])
                    nc.vector.tensor_add(dst, t1[:], t2[:])

                for nch in range(NCH):
                    c0 = nch * 512
                    xc = xc_next
                    if nch == DEFER_CHUNK:
                        xq_c1.extend(xc)
                    if nch + 1 < NCH:
                        xc_next = load_xc(nch + 1)
                    if nch == 0:
                        # second wqk column half (heads m>=5, first needed
                        # ~18us after the m=0 chain starts) queued on sync
                        # BEHIND xc0/wv/xc1 so it cannot crowd them out
                        for kt in range(NKT):
                            nc.sync.dma_start(
                                out=wqk_t[kt][:, half_w:],
                                in_=wqk[kt * P:(kt + 1) * P, half_w:])
                    # v projection first: depends only on xc + wv (2.5 MB),
                    # so PE starts before the full wqk lands
                    for tl_i in range(4):
                        tt = nch * 4 + tl_i
                        ps = pj_pool.tile([P, GDIM], f32, tag="pj")
                        for kt in range(NKT):
                            nc.tensor.matmul(
                                ps[:],
                                lhsT=xc[kt][:, tl_i * P:(tl_i + 1) * P],
                                rhs=wv_t[kt][:],
                                start=(kt == 0), stop=(kt == NKT - 1))
                        for g in range(NG):
                            nc.vector.tensor_copy(
                                v_aug[tt][:, g * (P + 1):g * (P + 1) + P],
                                ps[:, g * P:(g + 1) * P])
                    # q/k channel-major projection with fused rope eviction;
                    # q heads of DEFER_CHUNK are spliced into attn window 0.
                    # The last chunk runs k heads first so the final rope
                    # eviction isn't on the first QK matmul's critical path.
                    m_order = (list(range(NH, NH + NG)) + list(range(NH))
                               if nch == NCH - 1 else list(range(NH + NG)))
                    for m in m_order:
                        if nch == DEFER_CHUNK and m < NH:
                            continue
                        ps = pj_pool.tile([P, 512], f32, tag="pj")
                        for kt in range(NKT):
                            nc.tensor.matmul(
                                ps[:],
                                lhsT=wqk_t[kt][:, m * P:(m + 1) * P],
                                rhs=xc[kt][:],
                                start=(kt == 0), stop=(kt == NKT - 1))
                        rope_evict(ps, m, c0, 512, qkT[m][:, c0:c0 + 512])

                c1 = DEFER_CHUNK * 512
                nc.vector.tensor_copy(tabq2["cos"][:], tabs["cosq"][:, c1:c1 + 512])
                nc.vector.tensor_copy(tabq2["sin"][:], tabs["sinq"][:, c1:c1 + 512])

            # ------------- phase 2: attention with spliced o_proj ---------
            # PSUM budget (8 banks; a matmul accumulation chain owns a full
            # 2KB zero-region/bank): mmA 2 + mmB 1 + pv 4 + op 1.
            mm_pool = ctx.enter_context(tc.tile_pool(name="mm", bufs=1, space="PSUM"))
            pv_pool = ctx.enter_context(tc.tile_pool(name="pv", bufs=4, space="PSUM"))
            op_pool = ctx.enter_context(tc.tile_pool(name="op", bufs=1, space="PSUM"))
            attn_pool = ctx.enter_context(tc.tile_pool(name="attn", bufs=8))
            aT_pool = ctx.enter_context(tc.tile_pool(name="aT", bufs=1))
            rc_pool = ctx.enter_context(tc.tile_pool(name="rc", bufs=8))
            pt_pool = ctx.enter_context(tc.tile_pool(name="pt", bufs=3))
            osb_pool = ctx.enter_context(tc.tile_pool(name="osb", bufs=6))
            wo_pool = ctx.enter_context(tc.tile_pool(name="wo", bufs=NDT))
            wq2_pool = ctx.enter_context(tc.tile_pool(name="wq2", bufs=3))
            rt2_pool = ctx.enter_context(tc.tile_pool(name="rt2", bufs=1))
            # attn tiles are written in window qc and consumed (transposed)
            # in window qc+1, so 8 slots suffice: qt and qt+8 never overlap
            attn8 = [attn_pool.tile([P, QDIM], bf16, tag="attn", name=f"attn{i}")
                     for i in range(8)]
            attn_t = [attn8[qt % 8] for qt in range(NTT)]
            # d-major attention output, filled by DMA-engine transposes:
            # aTall[p, d, q] = attn_t[q % 512...][q-part, d*128+p]
            aTall = aT_pool.tile([P, NDT, T], bf16, tag="aT", name="aTall")

            wo_t = []
            for dtile in range(NDT):
                tl = wo_pool.tile([P, D_MODEL], bf16, tag="wo", name=f"wot{dtile}")
                nc.sync.dma_start(out=tl[:], in_=wo[dtile * P:(dtile + 1) * P, :])
                wo_t.append(tl)

            # --- splice-op factories (run between PV groups on the PE) ---
            # During attention windows psum chains rotate through the single
            # op bank; in the tail (pv banks free) a deeper rotation is
            # passed in so chains/evictions pipeline.
            def issue_transposes(qc, part):
                # part 0: heads 0..6 (issued right after head 6's eviction,
                # hiding the xbar latency); part 1: the final head's columns
                d0, d1 = (0, NDT - 1) if part == 0 else (NDT - 1, NDT)
                for j in range(4):
                    qt = qc * 4 + j
                    nc.sync.dma_start_transpose(
                        out=aTall[:, d0:d1, qt * P:(qt + 1) * P],
                        in_=attn_t[qt][:, d0 * P:d1 * P])

            def make_oproj_ops(qc, alloc=None):
                is_tail = alloc is not None
                alloc = alloc or [lambda: op_pool.tile([P, 512], f32, tag="op",
                                                       name="opps")]
                ops = []
                for j in range(4):
                    tt = qc * 4 + j
                    for nchn in range(NCH):
                        ci = j * NCH + nchn
                        state = {}
                        for dtile in range(NDT):
                            def op(tt=tt, nchn=nchn, dtile=dtile, state=state,
                                   ci=ci):
                                if dtile == 0:
                                    state["ps"] = alloc[ci % len(alloc)]()
                                ps = state["ps"]
                                nc.tensor.matmul(
                                    ps[:],
                                    lhsT=aTall[:, dtile, tt * P:(tt + 1) * P],
                                    rhs=wo_t[dtile][:, nchn * 512:(nchn + 1) * 512],
                                    start=(dtile == 0), stop=(dtile == NDT - 1))
                                if dtile == NDT - 1:
                                    osb = osb_pool.tile([P, 512], bf16, tag="osb")
                                    nc.vector.tensor_copy(osb[:], ps[:])
                                    if is_tail:
                                        # tail: split across two queues so the
                                        # final drain transfer is halved
                                        for hf in range(2):
                                            nc.sync.dma_start(
                                                out=out[tt * P:(tt + 1) * P,
                                                        nchn * 512 + hf * 256:
                                                        nchn * 512 + (hf + 1) * 256],
                                                in_=osb[:, hf * 256:(hf + 1) * 256])
                                    else:
                                        nc.sync.dma_start(
                                            out=out[tt * P:(tt + 1) * P,
                                                    nchn * 512:(nchn + 1) * 512],
                                            in_=osb[:])
                            ops.append(op)
                return ops

            def make_deferred_q_ops():
                # chunk-1 q-head projection + rope, spliced into window 0.
                # Weight slices stream from the repacked wqk2 via a small
                # rolling pool; rope reads the psum chain directly.
                c1 = DEFER_CHUNK * 512
                ops = []
                wtiles = {}

                def load(m):
                    tl = wq2_pool.tile([P, D_MODEL], bf16, tag="wq2", name=f"wq2_{m}")
                    nc.sync.dma_start(
                        out=tl[:], in_=wqk2[:, m * D_MODEL:(m + 1) * D_MODEL])
                    wtiles[m] = tl

                load(0)
                load(1)
                for m in range(NH):
                    state = {}
                    for kt in range(NKT):
                        def op(m=m, kt=kt, state=state):
                            if kt == 0:
                                state["ps"] = op_pool.tile(
                                    [P, 512], f32, tag="op", name="qps")
                                if m + 2 < NH:
                                    load(m + 2)
                            nc.tensor.matmul(
                                state["ps"][:],
                                lhsT=wtiles[m][:, kt * P:(kt + 1) * P],
                                rhs=xq_c1[kt][:],
                                start=(kt == 0), stop=(kt == NKT - 1))
                            if kt == NKT - 1:
                                ps = state["ps"]
                                t1 = rt2_pool.tile([P, 512], f32, tag="t1")
                                t2 = rt2_pool.tile([P, 512], f32, tag="t2")
                                hp = P // 2
                                nc.vector.tensor_mul(t1[:], ps[:], tabq2["cos"][:])
                                nc.vector.tensor_mul(t2[0:hp, :], ps[hp:P, :],
                                                     tabq2["sin"][0:hp, :])
                                nc.vector.tensor_mul(t2[hp:P, :], ps[0:hp, :],
                                                     tabq2["sin"][hp:P, :])
                                nc.vector.tensor_add(
                                    qkT[m][:, c1:c1 + 512], t1[:], t2[:])
                        ops.append(op)
                return ops

            from collections import deque
            splice_q = deque()
            splice_state = {"total": 0, "done": 0, "kts": 0}

            def push_ops(ops):
                splice_q.extend(ops)
                splice_state["total"] += len(ops)

            def splice(nkts):
                st = splice_state
                st["kts"] += nkts
                target = min(st["total"], (st["kts"] * st["total"] + 127) // 128)
                while st["done"] < target and splice_q:
                    splice_q.popleft()()
                    st["done"] += 1

            def window_reset():
                splice_state["total"] = len(splice_q)
                splice_state["done"] = 0
                splice_state["kts"] = 0

            cur_slots = None

            def emit_pv(h, kts, pt):
                nonlocal cur_slots
                g = h // 4
                for i, kt in enumerate(kts):
                    if kt == 0:
                        cur_slots = [pv_pool.tile([P, P + 4], f32, tag="pv",
                                                  name=f"pv{j}")
                                     for j in range(4)]
                    for j in range(4):
                        nc.tensor.matmul(
                            cur_slots[j][:, 0:P + 1],
                            lhsT=pt[:, i * 512 + j * P:i * 512 + (j + 1) * P],
                            rhs=v_aug[kt][:, g * (P + 1):(g + 1) * (P + 1)],
                            start=(kt == 0), stop=(kt == NTT - 1))

            def emit_evict(qc, h):
                for j in range(4):
                    qt = qc * 4 + j
                    rc = rc_pool.tile([P, 1], f32, tag="rc")
                    nc.vector.reciprocal(rc[:], cur_slots[j][:, P:P + 1])
                    nc.vector.tensor_scalar_mul(
                        attn_t[qt][:, h * P:(h + 1) * P],
                        cur_slots[j][:, 0:P], rc[:])

            def retire(pqc, ph, pkts, ppt):
                emit_pv(ph, pkts, ppt)
                if pkts[-1] == NTT - 1:
                    emit_evict(pqc, ph)
                    if ph == NH - 2:
                        issue_transposes(pqc, 0)
                    elif ph == NH - 1:
                        issue_transposes(pqc, 1)
                        if pqc < NCH - 1:  # last window's o_proj runs in the tail
                            push_ops(make_oproj_ops(pqc))
                        window_reset()

            push_ops(make_deferred_q_ops())
            window_reset()
            pending = deque()
            for qc in range(NCH):
                for h in range(NH):
                    for kind, kts in TILE_SCHED:
                        g = h // 4
                        w = 512 * len(kts)
                        ps = mm_pool.tile([P, w], f32, tag="mm" + kind,
                                          name="mm" + kind)
                        for i, kt in enumerate(kts):
                            nc.tensor.matmul(
                                ps[:, i * 512:(i + 1) * 512],
                                lhsT=qkT[NH + g][:, kt * P:(kt + 1) * P],
                                rhs=qkT[h][:, qc * 512:(qc + 1) * 512],
                                start=True, stop=True)
                        pt = pt_pool.tile([P, w], bf16, tag="pt" + kind)
                        if masked:
                            for i, kt in enumerate(kts):
                                nc.scalar.activation(
                                    pt[:, i * 512:(i + 1) * 512],
                                    ps[:, i * 512:(i + 1) * 512],
                                    AF.Exp, bias=maskcol_t[:, kt:kt + 1])
                        else:
                            nc.scalar.activation(pt[:], ps[:], AF.Exp)
                        # PV trails QK/EXP by two score-tiles so the first PV
                        # of a tile never waits on the exp semaphore
                        pending.append((qc, h, kts, pt))
                        if len(pending) > 2:
                            retire(*pending.popleft())
                        splice(len(kts))

            while pending:
                retire(*pending.popleft())
            while splice_q:  # any unfinished o_proj leftovers from window 2
                splice_q.popleft()()
            # tail: pv banks are free now — rotate chains over pv+op banks
            # so psum evictions overlap the next chain instead of stalling
            tail_alloc = (
                [lambda: op_pool.tile([P, 512], f32, tag="op", name="tl0")]
                + [lambda j=j: pv_pool.tile([P, 512], f32, tag="pv",
                                            name=f"tl{j}")
                   for j in range(1, 5)])
            for f in make_oproj_ops(NCH - 1, tail_alloc):
                f()

    nc.compile()
    return nc


def make_tables():
    inv_freq = 1.0 / (THETA ** (np.arange(0, HEAD_DIM, 2, dtype=np.float32)
                                / HEAD_DIM))          # [64]
    ang = np.arange(T, dtype=np.float32)[:, None] * inv_freq[None, :]  # [T, 64]
    cos = np.cos(ang).T.astype(np.float32)            # [64, T]
    sin = np.sin(ang).T.astype(np.float32)
    cos2 = np.concatenate([cos, cos], axis=0)         # [128, T]
    sinA = np.concatenate([-sin, sin], axis=0)        # [128, T]
    return (np.ascontiguousarray(cos2 * SCALE).astype(BF16),
            np.ascontiguousarray(sinA * SCALE).astype(BF16),
            np.ascontiguousarray(cos2).astype(BF16),
            np.ascontiguousarray(sinA).astype(BF16))


def make_in_maps(x, W_qkv, W_o, padding_mask, masked):
    cosq_v, sinq_v, cosk_v, sink_v = make_tables()
    in_maps = []
    for c in range(N_CORES):
        b, half = c // 2, c % 2
        q0 = half * QDIM
        k0 = NUM_HEADS * HEAD_DIM + half * GDIM
        v0 = NUM_HEADS * HEAD_DIM + QUERY_GROUPS * HEAD_DIM + half * GDIM
        wqk_v = np.concatenate(
            [W_qkv[:, q0:q0 + QDIM], W_qkv[:, k0:k0 + GDIM]], axis=1).astype(BF16)
        # repack q-head columns kt-major for the deferred chunk-1 projection
        wqk2_v = np.ascontiguousarray(
            wqk_v[:, :QDIM].reshape(NKT, P, NH, P).transpose(1, 2, 0, 3)
            .reshape(P, NH * D_MODEL))
        m = {
            "xt": np.ascontiguousarray(x[b].T).astype(BF16),
            "wqk": np.ascontiguousarray(wqk_v),
            "wqk2": wqk2_v,
            "wv": np.ascontiguousarray(W_qkv[:, v0:v0 + GDIM]).astype(BF16),
            "wo": np.ascontiguousarray(W_o[half * QDIM:(half + 1) * QDIM, :]).astype(BF16),
            "cosq": cosq_v, "sinq": sinq_v, "cosk": cosk_v, "sink": sink_v,
        }
        if masked:
            bias = np.where(padding_mask[b], 0.0, -1e30).astype(np.float32)  # [T]
            m["maskcol"] = np.ascontiguousarray(
                bias.reshape(NTT, P).T).astype(np.float32)
        in_maps.append(m)
    return in_maps


_nc_cache = {}


def kernel(x, W_qkv, W_o, padding_mask, trace=False):
    from concourse.bass_utils import run_bass_kernel_spmd

    x = np.asarray(x)
    W_qkv = np.asarray(W_qkv)
    W_o = np.asarray(W_o)
    padding_mask = np.asarray(padding_mask)
    masked = not bool(padding_mask.all())

    if masked not in _nc_cache:
        _nc_cache[masked] = build_nc(masked)
    nc = _nc_cache[masked]

    in_maps = make_in_maps(x, W_qkv, W_o, padding_mask, masked)
    res = run_bass_kernel_spmd(
        nc, in_maps, core_ids=list(range(N_CORES)),
        trace=trace, trace_cores=[0] if trace else None)

    out = np.empty((B, T, D_MODEL), np.float32)
    for b in range(B):
        out[b] = (res.results[2 * b]["out"].astype(np.float32)
                  + res.results[2 * b + 1]["out"].astype(np.float32))
    kernel.last_exec_time_ns = res.exec_time_ns
    kernel.last_results = res
    return out
